# revision 49
# baseline (speedup 1.0000x reference)
"""Chunked-prefill paged attention kernel for Trainium2 (Bass/Tile), 8 cores.

Sharding: tensor-parallel over heads. Core i handles q heads 4i..4i+3 and
kv head i. The paged-cache scatter/gather (index-driven data movement) is
resolved on the host; each core runs dense attention over the gathered
[ctx | chunk] keys/values for its kv head.

Per-core structure ("transposed scores"): loop over (q-chunk c, head-pair
hp); inner loop over 128-row l-tiles:
  - 2 QK^T matmuls (fp16, kv-head kT stationary shared) -> fp32 PSUM pair
    tile [128,2,512] (2 banks, double-buffered).
  - exp of both heads' scores, split across TWO engines per-tile: most on
    ACT (exp, scale=ln2/1024 -- q is host-prescaled by SCALE*log2e*1024),
    some on the otherwise-idle DVE via a custom 8-stage op computing
    z = X + a*(frac1024(X) + B)^2/1024 followed by a tensor_scalar
    (+K, f32->int16 convert) that lands the fp16 BITS of 2^(X/1024)
    (Schraudolph + quadratic correction, ~4e-3 max rel err on those tiles).
  - causal mask: DVE triangle multiply on diagonal blocks; gpsimd zeroes
    the invisible column prefix of partially-visible tiles.
  - 2 PV matmuls (fp16) accumulate into a [128,2,512] PSUM pair.
  - softmax denominators WITHOUT per-tile PE matmuls: a streaming binary
    add-tree over the exi tiles in fp16 on DVE (+ some nodes on gpsimd),
    finished by 2 ones-matmuls per pass ([1,512] each) on the PE.
The unnormalized oT and denominators are DMA'd out; the host divides and
transposes.
"""

import numpy as np

import concourse.bacc as bacc
import concourse.bass as bass
import concourse.mybir as mybir
import concourse.tile as tile
from concourse.bass_utils import run_bass_kernel_spmd

NH, NKVH, HD = 32, 8, 128
SCALE = 0.08838834764831845  # 1/sqrt(128)
LOG2E = 1.4426950408889634
LN2 = 0.6931471805599453
SEQ, CTX = 1024, 3072
L = CTX + SEQ  # 4096
NDEV = 8
HPD = NH // NDEV  # q heads per device
QCH = 512  # q columns per chunk (psum bank width in f32)
NQC = SEQ // QCH
NT = L // 128  # 32 l-tiles
NT_CTX = CTX // 128  # 24 context l-tiles

F32 = mybir.dt.float32
FP16 = mybir.dt.float16
I16 = mybir.dt.int16

# which st=0 tiles exp on the DVE instead of ACT; which tree-adds go to
# gpsimd instead of DVE
DVE_EXP_MOD = 9
POOL_ADD_MOD = 1000000000

# ---- custom DVE exp2-bits op -------------------------------------------
# fit of g(f) = 2^f - f on [0,1) by a*(f+b)^2 + c
A_FIT, B_FIT, C_FIT = 0.3426563535259007, -0.5115507103083531, 0.9140946377045693
EXP_S0 = 512.0          # floor bias (round(X-512) = 1024*floor(x))
EXP_S1 = 1.5 * 2**33    # magic round-to-multiple-of-1024 constant
EXP_IMM2 = 1024.0 * B_FIT
EXP_A3 = A_FIT / 1024.0  # via in1 latch
EXP_KADD = 1024.0 * (14.0 + C_FIT)  # added by the tensor_scalar step


def _exp2a_ref(in0, in1, c0, c1, c2):
    x = np.asarray(in0, np.float32)
    a3 = np.asarray(in1, np.float32).reshape(-1, 1)
    h = (x - np.float32(c0)).astype(np.float32)
    r = (h + np.float32(c1)).astype(np.float32)
    i = (r - np.float32(c1)).astype(np.float32)
    F = (x - i).astype(np.float32)
    u = (F + np.float32(c2)).astype(np.float32)
    v = (u * u).astype(np.float32)
    w = (v * a3).astype(np.float32)
    return (x + w).astype(np.float32)


def _register_exp2a():
    import concourse.dve_ops as dve_ops
    from concourse.dve_ops import DveOp
    from concourse.dve_spec import (
        C0,
        C1,
        C2,
        C3,
        Spec,
        Src0,
        _spill_c3_to_src1,
        lower,
    )
    from concourse.dve_uop import DveOpSpec

    for op in dve_ops.OPS:
        if op.name == "AntExp2A":
            return op
    x = Src0
    h = x - C0
    r = h + C1
    i = r - C1
    F = x - i
    u = F + C2
    v = u * u
    w = v * C3
    z = x + w
    spec = Spec(body=_spill_c3_to_src1(z), reference=_exp2a_ref)
    row = max(dve_ops._SUB_OPCODE_FOR_NAME.values()) + 1
    assert row < 0x20
    sha = DveOpSpec(
        name="AntExp2A", opcode=row, uops=lower(spec, ver="v3"), rd1_en=True
    ).sha("v3")
    op = DveOp(name="AntExp2A", spec=spec, subdim=False, uops_sha={"v3": sha})
    dve_ops.OPS.append(op)
    dve_ops._SUB_OPCODE_FOR_NAME[op.name] = row
    dve_ops.CUSTOM_DVE_SPECS[op.name] = spec
    return op


_CACHE = {}


def _tiles_for_chunk(c):
    """(lt, st, diag) per l-tile: st = first visible q-col, diag = needs
    triangular mask at cols [st, st+128)."""
    out = [(lt, 0, False) for lt in range(NT_CTX)]
    for b in range(4 * (c + 1)):
        st = 128 * b - QCH * c
        out.append((NT_CTX + b, max(st, 0), st >= 0))
    return out


def _build():
    exp_op = _register_exp2a()
    nc = bacc.Bacc("TRN2", target_bir_lowering=False, debug=False)

    NKC = 4
    TPC = NT // NKC  # l-tiles per load chunk
    # q head-pairs, host-prescaled by SCALE*log2e*1024, chunk-major:
    # [hp, HD, NQC, 2, QCH]
    qdT = nc.dram_tensor(
        "qdT", [HPD // 2, HD, NQC, 2, QCH], FP16, kind="ExternalInput"
    )
    kdT = nc.dram_tensor("kdT", [NKC, HD, L // NKC], FP16, kind="ExternalInput")
    vd = nc.dram_tensor("vd", [NKC, HD, TPC, HD], FP16, kind="ExternalInput")
    tri = nc.dram_tensor("tri", [128, 128], FP16, kind="ExternalInput")
    # micro-lead blob gating the first matmuls: [kT l-tiles 0-3 |
    # qpair hp=0 c=0 (both heads)] in one DMA
    lead = nc.dram_tensor("lead", [HD, 3, QCH], FP16, kind="ExternalInput")
    od = nc.dram_tensor(
        "od", [NQC, HPD // 2, HD, 2, QCH], FP16, kind="ExternalOutput"
    )
    sums_out = nc.dram_tensor(
        "sums", [NQC * HPD // 2, 2, QCH], F32, kind="ExternalOutput"
    )

    with tile.TileContext(nc) as tc:
        with (
            tc.tile_pool(name="inp", bufs=1) as inp,
            tc.tile_pool(name="small", bufs=1) as small,
            tc.tile_pool(name="exq", bufs=8) as exq,
            tc.tile_pool(name="zsp", bufs=3) as zsp,
            tc.tile_pool(name="tsp", bufs=3) as tsp,
            tc.tile_pool(name="ssb", bufs=2) as ssb,
            tc.tile_pool(name="osb", bufs=2) as osb,
            tc.tile_pool(name="scp", bufs=2, space="PSUM") as scp,
            tc.tile_pool(name="accps", bufs=1, space="PSUM") as accps,
            tc.tile_pool(name="sumps", bufs=1, space="PSUM") as sumps,
        ):
            # ---- input loads first (lead gates the first matmuls);
            # two separate tiles on two queues: the first QK (head 0) is
            # gated only by lead_a, head 1 by lead_b ----
            lead_a = inp.tile([128, 2, QCH], FP16, name="lead_a", tag="lead_a")
            lead_b = inp.tile([128, QCH], FP16, name="lead_b", tag="lead_b")
            nc.sync.dma_start(out=lead_a, in_=lead[:, 0:2, :])
            nc.scalar.dma_start(out=lead_b, in_=lead[:, 2, :])

            # ---- constants ----
            tri_sb = small.tile([128, 128], FP16, tag="tri")
            ones_f = small.tile([128, 1], F32, tag="ones_f")
            nc.vector.memset(ones_f, 1.0)
            ones_h = small.tile([128, 1], FP16, tag="ones")
            nc.vector.tensor_copy(out=ones_h, in_=ones_f)
            a3_sb = small.tile([128, 1], F32, tag="a3")
            nc.vector.memset(a3_sb, EXP_A3)

            kT = [
                inp.tile([128, L // NKC], FP16, name=f"kT{i}", tag=f"kT{i}")
                for i in range(NKC)
            ]
            qT = [
                inp.tile([128, NQC, 2, QCH], FP16, name=f"qT{h}", tag=f"qT{h}")
                for h in range(HPD // 2)
            ]
            v_h = [
                inp.tile([128, TPC, HD], FP16, name=f"v{i}", tag=f"v{i}")
                for i in range(NKC)
            ]

            # ordered by first use: v0 (PV of tile 0), kT chunks (tiles
            # 8/16/24), qT1 (pass 1) before qT0 (pass 2), tri (tile 24)
            nc.scalar.dma_start(out=v_h[0], in_=vd[0, :, :, :])
            nc.sync.dma_start(out=kT[0], in_=kdT[0, :, :])
            nc.scalar.dma_start(out=v_h[1], in_=vd[1, :, :, :])
            for i in range(1, NKC):
                nc.sync.dma_start(out=kT[i], in_=kdT[i, :, :])
            nc.scalar.dma_start(out=v_h[2], in_=vd[2, :, :, :])
            nc.sync.dma_start(out=qT[1], in_=qdT[1, :, :, :, :])
            nc.scalar.dma_start(out=v_h[3], in_=vd[3, :, :, :])
            nc.sync.dma_start(out=qT[0], in_=qdT[0, :, :, :, :])
            nc.scalar.dma_start(out=tri_sb, in_=tri[:, :])

            def kT_at(lt):
                if lt < 4:
                    return lead_a[:, 0, lt * 128 : (lt + 1) * 128]
                i, o = lt // TPC, (lt % TPC) * 128
                return kT[i][:, o : o + 128]

            def v_at(lt):
                return v_h[lt // TPC][:, lt % TPC, :]

            assert TPC == 8  # kT_at/v_at index by lt // TPC

            # ---- main: flat software pipeline across all 4 passes ----
            passes = [
                (c, hp, _tiles_for_chunk(c))
                for c in range(NQC)
                for hp in range(HPD // 2)
            ]
            flat = [
                (p, i)
                for p, (_, _, tiles) in enumerate(passes)
                for i in range(len(tiles))
            ]
            ctx = {}  # pass idx -> dict(acc, sums_ps, ex, stack)

            def emit_sum_reduce(cp, node):
                """Accumulate one subtree's column sums into sums_ps (PE)."""
                start = cp["sums_ps"] is None
                if start:
                    cp["sums_ps"] = sumps.tile(
                        [1, 2, QCH], F32, name="sums_ps", tag="sums_ps"
                    )
                for j in range(2):
                    nc.tensor.matmul(
                        cp["sums_ps"][:, j, :],
                        ones_h,
                        node[:, j, :],
                        start=start,
                        stop=True,
                        skip_group_check=True,
                    )

            def tree_push(cp, node, offload_l1=False):
                """Streaming binary tree over exi tiles (fp16), capped at
                16-tile subtrees; each subtree root goes straight to the
                PE denominator accumulation (2 tiny matmuls). When the NEXT
                tile exps on the DVE, the lvl-1 combine goes to gpsimd so
                the DVE queue is clear for the custom exp (its consumer is
                tiles away, so the slow Pool engine is fine)."""
                lvl = 0
                while cp["stack"] and cp["stack"][-1][0] == lvl:
                    plvl, ptile = cp["stack"].pop()
                    out = tsp.tile(
                        [128, 2, QCH], FP16, name=f"ts{lvl + 1}",
                        tag=f"ts{lvl + 1}",
                    )
                    eng = nc.gpsimd if (offload_l1 and lvl == 0) else nc.vector
                    eng.tensor_add(out=out, in0=ptile, in1=node)
                    node = out
                    lvl += 1
                    if lvl == 4:
                        emit_sum_reduce(cp, node)
                        return
                cp["stack"].append((lvl, node))

            def emit_qk_exp(p, i):
                c, hp, tiles = passes[p]
                if i == 0:
                    ctx[p] = {
                        "acc": accps.tile([128, 2, QCH], F32, name="acc", tag="acc"),
                        "ex": [None] * len(tiles),
                        "stack": [],
                        "sums_ps": None,
                    }
                cp = ctx[p]
                lt, st, diag = tiles[i]
                pair = scp.tile([128, 2, QCH], F32, name="pair", tag="pair")
                for j in range(2):
                    if c == 0 and hp == 0:
                        qmv = lead_a[:, 1, st:] if j == 0 else lead_b[:, st:]
                    else:
                        qmv = qT[hp][:, c, j, st:]
                    nc.tensor.matmul(
                        pair[:, j, st:], kT_at(lt), qmv, start=True, stop=True
                    )
                exi = exq.tile([128, 2, QCH], FP16, name="exi", tag="ex")
                if st > 0:
                    # zero the invisible prefix so the tree-add is exact
                    # (emitted before exp: no dependency, runs early)
                    nc.vector.memset(exi[:, :, 0:st], 0)
                if st == 0 and not diag and (i % DVE_EXP_MOD) == 2:
                    zs = zsp.tile([128, 2, QCH], F32, name="zs", tag="zs")
                    nc.vector._custom_dve(
                        exp_op,
                        out=zs,
                        in0=pair,
                        in1=a3_sb,
                        s0=EXP_S0,
                        s1=EXP_S1,
                        imm2=EXP_IMM2,
                    )
                    nc.vector.tensor_scalar(
                        out=exi.bitcast(I16),
                        in0=zs,
                        scalar1=EXP_KADD,
                        scalar2=None,
                        op0=mybir.AluOpType.add,
                    )
                else:
                    nc.scalar.activation(
                        out=exi[:, :, st:],
                        in_=pair[:, :, st:],
                        func=mybir.ActivationFunctionType.Exp,
                        scale=LN2 / 1024.0,
                    )
                    if diag:
                        for j in range(2):
                            nc.vector.tensor_mul(
                                out=exi[:, j, st : st + 128],
                                in0=exi[:, j, st : st + 128],
                                in1=tri_sb,
                            )
                cp["ex"][i] = exi
                tree_push(cp, exi)

            def emit_pv(p, i):
                c, hp, tiles = passes[p]
                cp = ctx[p]
                last_i = len(tiles) - 1
                lt, st, diag = tiles[i]
                exi = cp["ex"][i]
                for j in range(2):
                    nc.tensor.matmul(
                        cp["acc"][:, j, st:],
                        v_at(lt),
                        exi[:, j, st:],
                        start=(i == 0),
                        stop=(i == last_i),
                        skip_group_check=True,
                    )
                if i == last_i:
                    # collapse the leftover stack into one node and fold it
                    # into the PE denominator accumulation; the drain DMAs
                    # are deferred a few tiles so the PE queue stays fed
                    stack = cp["stack"]
                    while len(stack) > 1:
                        l1, t1 = stack.pop()
                        l0, t0 = stack.pop()
                        out = tsp.tile(
                            [128, 2, QCH], FP16, name=f"tf{l0}", tag=f"ts{l0 + 1}"
                        )
                        nc.vector.tensor_add(out=out, in0=t0, in1=t1)
                        stack.append((l0 + 1, out))
                    if stack:
                        emit_sum_reduce(cp, stack.pop()[1])

            def emit_drains(p):
                c, hp, tiles = passes[p]
                cp = ctx[p]
                pidx = c * (HPD // 2) + hp
                # deferred past the subtree folds, so these copies never
                # head-of-line-block their queues
                sums_sb = ssb.tile([1, 2, QCH], F32, tag="sums_sb")
                if p == len(passes) - 1:
                    nc.scalar.copy(out=sums_sb[:, 0, :], in_=cp["sums_ps"][:, 0, :])
                    nc.vector.tensor_copy(
                        out=sums_sb[:, 1, :], in_=cp["sums_ps"][:, 1, :]
                    )
                else:
                    nc.scalar.copy(out=sums_sb, in_=cp["sums_ps"])
                nc.sync.dma_start(out=sums_out[pidx, :, :], in_=sums_sb)
                acc_sb = osb.tile([128, 2, QCH], FP16, tag="acc_sb")
                if p == len(passes) - 1:
                    # last pass: split across engines + DMAs for a short tail
                    nc.vector.tensor_copy(out=acc_sb[:, 0, :], in_=cp["acc"][:, 0, :])
                    nc.sync.dma_start(out=od[c, hp, :, 0, :], in_=acc_sb[:, 0, :])
                    nc.scalar.copy(out=acc_sb[:, 1, :], in_=cp["acc"][:, 1, :])
                    nc.scalar.dma_start(out=od[c, hp, :, 1, :], in_=acc_sb[:, 1, :])
                else:
                    nc.vector.tensor_copy(out=acc_sb[:, 0, :], in_=cp["acc"][:, 0, :])
                    nc.vector.tensor_copy(out=acc_sb[:, 1, :], in_=cp["acc"][:, 1, :])
                    nc.sync.dma_start(out=od[c, hp, :, :, :], in_=acc_sb)

            LAG = 4
            DRAIN_DEFER = 3
            pending = []  # (due_w, pass_idx)
            for w in range(len(flat) + LAG + DRAIN_DEFER):
                while pending and pending[0][0] <= w:
                    emit_drains(pending.pop(0)[1])
                if w < len(flat):
                    emit_qk_exp(*flat[w])
                if w >= LAG and w - LAG < len(flat):
                    pp, ii = flat[w - LAG]
                    emit_pv(pp, ii)
                    if ii == len(passes[pp][2]) - 1:
                        pending.append((w + DRAIN_DEFER, pp))
            while pending:
                emit_drains(pending.pop(0)[1])
    nc.compile()
    return nc


def _prep_host(q, k, v, k_cache, v_cache, slot_mapping, context_slots):
    """Resolve the paged-cache scatter+gather on the host."""
    kh = np.ascontiguousarray(k).reshape(SEQ, NKVH, HD)
    vh = np.ascontiguousarray(v).reshape(SEQ, NKVH, HD)
    sm = np.asarray(slot_mapping)
    cs = np.asarray(context_slots)

    k_ctx = np.asarray(k_cache)[cs].copy()
    v_ctx = np.asarray(v_cache)[cs].copy()
    order = np.argsort(sm, kind="stable")
    ss = sm[order]
    j = np.searchsorted(ss, cs)
    jc = np.minimum(j, len(ss) - 1)
    hit = ss[jc] == cs
    if hit.any():
        src = order[jc[hit]]
        k_ctx[hit] = kh[src]
        v_ctx[hit] = vh[src]

    k_all = np.concatenate([k_ctx, kh], axis=0)  # [L, NKVH, HD]
    v_all = np.concatenate([v_ctx, vh], axis=0)
    return k_all, v_all


# results of the last run (exec time etc), for the local test harness
last_results = None

QPRE = np.float32(SCALE * LOG2E * 1024.0)


def kernel(q, k, v, k_cache, v_cache, slot_mapping, context_slots):
    global last_results
    q = np.asarray(q, dtype=np.float32)
    k_all, v_all = _prep_host(
        q, np.asarray(k), np.asarray(v), k_cache, v_cache,
        slot_mapping, context_slots,
    )

    if "nc" not in _CACHE:
        _CACHE["nc"] = _build()
    nc = _CACHE["nc"]

    tri = np.where(
        np.arange(128)[None, :] >= np.arange(128)[:, None], 1.0, 0.0
    ).astype(np.float16)

    in_maps = []
    for d in range(NDEV):
        qs = (q[:, d * HPD * HD : (d + 1) * HPD * HD] * QPRE).astype(np.float16)
        # [hp, HD, NQC, 2, QCH]
        qp = np.ascontiguousarray(
            qs.T.reshape(HPD // 2, 2, HD, NQC, QCH).transpose(0, 2, 3, 1, 4)
        )
        in_maps.append(
            {
                "qdT": qp,
                "lead": np.ascontiguousarray(
                    np.concatenate(
                        [
                            k_all[0:QCH, d, :].T[:, None, :].astype(np.float16),
                            qp[0][:, 0],
                        ],
                        axis=1,
                    )
                ),
                # [NKC, HD, L//NKC]: contiguous per-chunk kT blocks
                "kdT": np.ascontiguousarray(
                    k_all[:, d, :].T.reshape(HD, 4, L // 4).transpose(1, 0, 2)
                ).astype(np.float16),
                # [NKC, 128, TPC, HD]: partition p holds v[tile*128+p, :]
                "vd": np.ascontiguousarray(
                    v_all[:, d, :].reshape(4, 8, 128, HD).transpose(0, 2, 1, 3)
                ).astype(np.float16),
                "tri": tri,
            }
        )

    res = run_bass_kernel_spmd(nc, in_maps, core_ids=list(range(NDEV)))
    last_results = res

    out = np.empty((SEQ, NH * HD), dtype=np.float32)
    for d in range(NDEV):
        odr = res.results[d]["od"].astype(np.float32)
        # od: [NQC, HPD//2, HD, 2, QCH] -> oT [HPD, HD, SEQ]
        oT = odr.transpose(1, 3, 2, 0, 4).reshape(HPD, HD, SEQ)
        sb = res.results[d]["sums"]  # [NQC*HPD//2, 2, QCH]
        sums = np.empty((HPD, SEQ), dtype=np.float32)
        for c in range(NQC):
            for hp in range(HPD // 2):
                blk = sb[c * (HPD // 2) + hp]
                for j in range(2):
                    sums[2 * hp + j, c * QCH : (c + 1) * QCH] = blk[j]
        o = oT / sums[:, None, :]
        out[:, d * HPD * HD : (d + 1) * HPD * HD] = (
            o.transpose(2, 0, 1).reshape(SEQ, HPD * HD)
        )
    return out
